# revision 4
# baseline (speedup 1.0000x reference)
import numpy as np
import ml_dtypes

import concourse.bass as bass
import concourse.mybir as mybir
import concourse.tile as tile
from concourse.bass_utils import run_bass_kernel_spmd

B, C, HH, WW = 8, 256, 96, 96
N = HH * WW
P = 128
NT = N // 512
KB = 36
f32 = mybir.dt.float32
bf16 = mybir.dt.bfloat16
AF = mybir.ActivationFunctionType
AX = mybir.AxisListType
ALPHA = 0.01
DMA = "gpsimd"

_cached = {}


def _build():
    nc = bass.Bass()
    dma = getattr(nc, DMA)

    qb_d = nc.dram_tensor("qb", [C, N], bf16, kind="ExternalInput")
    vb_d = nc.dram_tensor("vb", [C, N], bf16, kind="ExternalInput")
    wqt_d = nc.dram_tensor("wqt", [C, C], bf16, kind="ExternalInput")
    wkt_d = nc.dram_tensor("wkt", [C, C], bf16, kind="ExternalInput")
    wvt_d = nc.dram_tensor("wvt", [C, C], bf16, kind="ExternalInput")
    wo1t_d = nc.dram_tensor("wo1t", [C, C], bf16, kind="ExternalInput")
    wo2t_d = nc.dram_tensor("wo2t", [C, C], bf16, kind="ExternalInput")
    bqb_d = nc.dram_tensor("bqb", [P, C], f32, kind="ExternalInput")
    bkb_d = nc.dram_tensor("bkb", [P, C], f32, kind="ExternalInput")
    bv_d = nc.dram_tensor("bv", [C], f32, kind="ExternalInput")
    bns_d = nc.dram_tensor("bns", [C], f32, kind="ExternalInput")
    bnt_d = nc.dram_tensor("bnt", [C], f32, kind="ExternalInput")
    bo1_d = nc.dram_tensor("bo1", [C], f32, kind="ExternalInput")
    bo2_d = nc.dram_tensor("bo2", [C], f32, kind="ExternalInput")
    id_d = nc.dram_tensor("ident", [P, P], f32, kind="ExternalInput")
    out_d = nc.dram_tensor("out", [C, N], f32, kind="ExternalOutput")

    with tile.TileContext(nc) as tc:
        with (
            tc.tile_pool(name="wpool", bufs=1) as wp,
            tc.tile_pool(name="vpool", bufs=1) as vp,
            tc.tile_pool(name="spool", bufs=1) as sp,
        ):
            wqt = [wp.tile([P, C], bf16, name=f"wqt{i}") for i in range(2)]
            wkt = [wp.tile([P, C], bf16, name=f"wkt{i}") for i in range(2)]
            wvt = [wp.tile([P, C], bf16, name=f"wvt{i}") for i in range(2)]
            wo1t = [wp.tile([P, C], bf16, name=f"wo1t{i}") for i in range(2)]
            wo2t = [wp.tile([P, C], bf16, name=f"wo2t{i}") for i in range(2)]
            for i in range(2):
                dma.dma_start(wqt[i][:], wqt_d[i * P:(i + 1) * P, :])
                dma.dma_start(wkt[i][:], wkt_d[i * P:(i + 1) * P, :])
                dma.dma_start(wvt[i][:], wvt_d[i * P:(i + 1) * P, :])
                dma.dma_start(wo1t[i][:], wo1t_d[i * P:(i + 1) * P, :])
                dma.dma_start(wo2t[i][:], wo2t_d[i * P:(i + 1) * P, :])
            bqb = wp.tile([P, C], f32, name="bqb")
            bkb = wp.tile([P, C], f32, name="bkb")
            dma.dma_start(bqb[:], bqb_d[:])
            dma.dma_start(bkb[:], bkb_d[:])
            vec = {}
            for nm, d in (("bv", bv_d), ("bns", bns_d), ("bnt", bnt_d),
                          ("bo1", bo1_d), ("bo2", bo2_d)):
                vec[nm] = [wp.tile([P, 1], f32, name=f"{nm}{i}") for i in range(2)]
                for i in range(2):
                    dma.dma_start(vec[nm][i][:], d[i * P:(i + 1) * P, None])
            ident = wp.tile([P, P], f32, name="ident")
            dma.dma_start(ident[:], id_d[:])

            value = [vp.tile([P, N], bf16, name=f"value{i}") for i in range(2)]
            attnT = [sp.tile([P, C], bf16, name=f"attnT{i}") for i in range(2)]

            with (
                tc.tile_pool(name="ps_s", bufs=1, space="PSUM") as ps_s,
                tc.tile_pool(name="xin", bufs=3) as xp,
                tc.tile_pool(name="qk", bufs=3) as qkp,
                tc.tile_pool(name="ps_a", bufs=2, space="PSUM") as psa,
            ):
                psum_s = [ps_s.tile([P, C], f32, name=f"psum_s{i}") for i in range(2)]
                prev = None

                def emit_scores(pair, nch):
                    qT, kT = pair
                    for cq in range(2):
                        nc.tensor.matmul(
                            psum_s[cq][:],
                            qT[:, cq * P:(cq + 1) * P], kT[:],
                            start=(nch == 0), stop=(nch == 71),
                            skip_group_check=True)

                for t in range(NT):
                    xq = [xp.tile([P, 512], bf16, name=f"xq{i}", tag=f"xq{i}")
                          for i in range(2)]
                    xv = [xp.tile([P, 512], bf16, name=f"xv{i}", tag=f"xv{i}")
                          for i in range(2)]
                    for i in range(2):
                        dma.dma_start(xq[i][:], qb_d[i * P:(i + 1) * P,
                                                     t * 512:(t + 1) * 512])
                        dma.dma_start(xv[i][:], vb_d[i * P:(i + 1) * P,
                                                     t * 512:(t + 1) * 512])
                    for d in range(2):
                        pv = psa.tile([P, 512], f32, name="pv", tag="pv")
                        nc.tensor.matmul(pv[:], wvt[0][:, d * P:(d + 1) * P],
                                         xq[0][:], start=True, stop=False)
                        nc.tensor.matmul(pv[:], wvt[1][:, d * P:(d + 1) * P],
                                         xq[1][:], start=False, stop=True)
                        nc.scalar.activation(value[d][:, t * 512:(t + 1) * 512],
                                             pv[:], AF.Identity,
                                             bias=vec["bv"][d][:])
                    for j in range(4):
                        nch = t * 4 + j
                        pq = psa.tile([P, C], f32, name="pq", tag="pq")
                        nc.tensor.matmul(pq[:], xq[0][:, j * P:(j + 1) * P],
                                         wqt[0][:], start=True, stop=False)
                        nc.tensor.matmul(pq[:], xq[1][:, j * P:(j + 1) * P],
                                         wqt[1][:], start=False, stop=True)
                        pk = psa.tile([P, C], f32, name="pk", tag="pk")
                        nc.tensor.matmul(pk[:], xv[0][:, j * P:(j + 1) * P],
                                         wkt[0][:], start=True, stop=False)
                        nc.tensor.matmul(pk[:], xv[1][:, j * P:(j + 1) * P],
                                         wkt[1][:], start=False, stop=True)
                        qT = qkp.tile([P, C], bf16, name="qT", tag="qT")
                        kT = qkp.tile([P, C], bf16, name="kT", tag="kT")
                        nc.vector.tensor_add(qT[:], pq[:], bqb[:])
                        nc.vector.tensor_add(kT[:], pk[:], bkb[:])
                        if prev is not None:
                            emit_scores(prev, nch - 1)
                        prev = (qT, kT)
                emit_scores(prev, 71)

                sx = qkp
                pst = psa
                attn2 = []
                for cq in range(2):
                    negmax = sx.tile([P, 1], f32, name="negmax", tag=f"nm{cq}")
                    nc.vector.reduce_max(negmax[:], psum_s[cq][:], axis=AX.X,
                                         negate=True)
                    att = sx.tile([P, C], f32, name="att", tag=f"att{cq}")
                    sume = sx.tile([P, 1], f32, name="sume", tag=f"se{cq}")
                    nc.scalar.activation(att[:], psum_s[cq][:], AF.Exp,
                                         bias=negmax[:], accum_out=sume[:])
                    recip = sx.tile([P, 1], f32, name="recip", tag=f"rc{cq}")
                    nc.vector.reciprocal(recip[:], sume[:])
                    a2 = sx.tile([P, C], f32, name="a2", tag=f"a2{cq}")
                    nc.scalar.activation(a2[:], att[:], AF.Copy, scale=recip[:])
                    attn2.append(a2)
                for j in range(2):
                    for i in range(2):
                        nc.tensor.transpose(psum_s[j][:, i * P:(i + 1) * P],
                                            attn2[i][:, j * P:(j + 1) * P],
                                            ident[:])
                    nc.vector.tensor_copy(attnT[j][:], psum_s[j][:])

            with (
                tc.tile_pool(name="yb", bufs=2) as yb,
                tc.tile_pool(name="ps_b", bufs=2, space="PSUM") as psb,
            ):
                vv = [value[d][:].rearrange("p (c k) -> p c k", k=KB)
                      for d in range(2)]
                for kk in range(NT):
                    ys = []
                    for cp in range(2):
                        po = psb.tile([P, 512], f32, name="po", tag=f"po{cp}")
                        for ki in range(2):
                            k = 2 * kk + ki
                            for d in range(2):
                                nc.tensor.matmul(
                                    po[:, ki * C:(ki + 1) * C],
                                    vv[d][:, cp * P:(cp + 1) * P, k],
                                    attnT[d][:],
                                    start=(d == 0), stop=(d == 1))
                        y = yb.tile([P, 512], bf16, name="y", tag=f"y{cp}")
                        nc.scalar.activation(y[:], po[:], AF.Lrelu,
                                             bias=vec["bnt"][cp][:],
                                             scale=vec["bns"][cp][:],
                                             alpha=ALPHA)
                        ys.append(y)
                    hs = []
                    for o in range(2):
                        ph = psb.tile([P, 512], f32, name="ph", tag=f"ph{o}", bufs=1)
                        nc.tensor.matmul(ph[:], wo1t[0][:, o * P:(o + 1) * P],
                                         ys[0][:], start=True, stop=False)
                        nc.tensor.matmul(ph[:], wo1t[1][:, o * P:(o + 1) * P],
                                         ys[1][:], start=False, stop=True)
                        h = yb.tile([P, 512], bf16, name="h", tag=f"h{o}")
                        nc.scalar.activation(h[:], ph[:], AF.Lrelu,
                                             bias=vec["bo1"][o][:], alpha=ALPHA)
                        hs.append(h)
                    for o2 in range(2):
                        pf = psb.tile([P, 512], f32, name="pf", tag=f"pf{o2}", bufs=1)
                        nc.tensor.matmul(pf[:], wo2t[0][:, o2 * P:(o2 + 1) * P],
                                         hs[0][:], start=True, stop=False)
                        nc.tensor.matmul(pf[:], wo2t[1][:, o2 * P:(o2 + 1) * P],
                                         hs[1][:], start=False, stop=True)
                        ob = yb.tile([P, 512], f32, name="ob", tag=f"ob{o2}")
                        nc.vector.tensor_scalar(ob[:], pf[:], vec["bo2"][o2][:],
                                                None, op0=mybir.AluOpType.add)
                        dma.dma_start(out_d[o2 * P:(o2 + 1) * P,
                                            kk * 512:(kk + 1) * 512], ob[:])
    return nc


def _split_waits(nc):
    for f in nc.m.functions:
        for bb in f.blocks:
            new = []
            for inst in bb.instructions:
                si = inst.sync_info
                if (si is not None and si.on_wait and len(si.on_wait) > 1
                        and not isinstance(inst, (mybir.InstNoOp,
                                                  mybir.InstEventSemaphore))):
                    for wi, w in enumerate(si.on_wait[:-1]):
                        new.append(mybir.InstNoOp(
                            name=f"{inst.name}-ws{wi}",
                            ins=[], outs=[],
                            engine=inst.engine,
                            sync_info=mybir.SyncInfo(on_wait=[w], on_update=[]),
                            bass_nofuse=True,
                        ))
                    inst.sync_info = mybir.SyncInfo(on_wait=[si.on_wait[-1]],
                                                    on_update=list(si.on_update))
                new.append(inst)
            bb.instructions[:] = new


def _prep(inputs):
    f = np.float32
    bb = ml_dtypes.bfloat16
    scale = f(1.0) / f(np.sqrt(N))
    wqt = (inputs["Wq"].T.astype(f) * scale).astype(bb)
    wkt = inputs["Wk"].T.astype(f).astype(bb)
    wvt = inputs["Wv"].T.astype(f).astype(bb)
    wo1t = inputs["Wo1"].T.astype(f).astype(bb)
    wo2t = inputs["Wo2"].T.astype(f).astype(bb)
    bqb = np.tile((inputs["bq"].astype(f) * scale)[None, :], (P, 1)).astype(f)
    bkb = np.tile(inputs["bk"].astype(f)[None, :], (P, 1)).astype(f)
    bns = (inputs["bn_gamma"].astype(f)
           / np.sqrt(inputs["bn_var"].astype(f) + np.float32(1e-4))).astype(f)
    bnt = (inputs["bn_beta"].astype(f)
           - inputs["bn_mean"].astype(f) * bns).astype(f)
    common = {
        "wqt": np.ascontiguousarray(wqt), "wkt": np.ascontiguousarray(wkt),
        "wvt": np.ascontiguousarray(wvt), "wo1t": np.ascontiguousarray(wo1t),
        "wo2t": np.ascontiguousarray(wo2t),
        "bqb": bqb, "bkb": bkb,
        "bv": inputs["bv"].astype(f), "bns": bns, "bnt": bnt,
        "bo1": inputs["bo1"].astype(f), "bo2": inputs["bo2"].astype(f),
        "ident": np.eye(P, dtype=f),
    }
    q = np.asarray(inputs["q"], dtype=f).reshape(B, C, N).astype(bb)
    v = np.asarray(inputs["v"], dtype=f).reshape(B, C, N).astype(bb)
    in_maps = []
    for b in range(B):
        m = dict(common)
        m["qb"] = np.ascontiguousarray(q[b])
        m["vb"] = np.ascontiguousarray(v[b])
        in_maps.append(m)
    return in_maps


def kernel(_trace=False, **inputs):
    if "nc" not in _cached:
        nc = _build()
        _split_waits(nc)
        _cached["nc"] = nc
    nc = _cached["nc"]
    in_maps = _prep(inputs)
    res = run_bass_kernel_spmd(nc, in_maps, core_ids=list(range(B)),
                               trace=_trace)
    out = np.stack([res.results[b]["out"] for b in range(B)], axis=0)
    if _trace:
        kernel.last_results = res
    return out.reshape(B, C, HH, WW).astype(np.float32)


# revision 6
# speedup vs baseline: 29807.7207x; 29807.7207x over previous
import numpy as np
import ml_dtypes

import concourse.bass as bass
import concourse.mybir as mybir
import concourse.tile as tile
from concourse.bass_utils import run_bass_kernel_spmd

B, C, HH, WW = 8, 256, 96, 96
N = HH * WW
P = 128
NT = N // 512
KB = 36
f32 = mybir.dt.float32
bf16 = mybir.dt.bfloat16
AF = mybir.ActivationFunctionType
AX = mybir.AxisListType
ALPHA = 0.01
DMA = "gpsimd"

_cached = {}


def _build():
    nc = bass.Bass()
    dma = getattr(nc, DMA)

    qb_d = nc.dram_tensor("qb", [C, N], bf16, kind="ExternalInput")
    vb_d = nc.dram_tensor("vb", [C, N], bf16, kind="ExternalInput")
    wqt_d = nc.dram_tensor("wqt", [C, C], bf16, kind="ExternalInput")
    wkt_d = nc.dram_tensor("wkt", [C, C], bf16, kind="ExternalInput")
    wvt_d = nc.dram_tensor("wvt", [C, C], bf16, kind="ExternalInput")
    wo1t_d = nc.dram_tensor("wo1t", [C, C], bf16, kind="ExternalInput")
    wo2t_d = nc.dram_tensor("wo2t", [C, C], bf16, kind="ExternalInput")
    bqb_d = nc.dram_tensor("bqb", [P, C], f32, kind="ExternalInput")
    bkb_d = nc.dram_tensor("bkb", [P, C], f32, kind="ExternalInput")
    bv_d = nc.dram_tensor("bv", [C], f32, kind="ExternalInput")
    bns_d = nc.dram_tensor("bns", [C], f32, kind="ExternalInput")
    bnt_d = nc.dram_tensor("bnt", [C], f32, kind="ExternalInput")
    bo1_d = nc.dram_tensor("bo1", [C], f32, kind="ExternalInput")
    bo2_d = nc.dram_tensor("bo2", [C], f32, kind="ExternalInput")
    id_d = nc.dram_tensor("ident", [P, P], f32, kind="ExternalInput")
    out_d = nc.dram_tensor("out", [C, N], f32, kind="ExternalOutput")

    with tile.TileContext(nc) as tc:
        with (
            tc.tile_pool(name="wpool", bufs=1) as wp,
            tc.tile_pool(name="vpool", bufs=1) as vp,
            tc.tile_pool(name="spool", bufs=1) as sp,
        ):
            wqt = [wp.tile([P, C], bf16, name=f"wqt{i}") for i in range(2)]
            wkt = [wp.tile([P, C], bf16, name=f"wkt{i}") for i in range(2)]
            wvt = [wp.tile([P, C], bf16, name=f"wvt{i}") for i in range(2)]
            wo1t = [wp.tile([P, C], bf16, name=f"wo1t{i}") for i in range(2)]
            wo2t = [wp.tile([P, C], bf16, name=f"wo2t{i}") for i in range(2)]
            for i in range(2):
                dma.dma_start(wqt[i][:], wqt_d[i * P:(i + 1) * P, :])
                dma.dma_start(wkt[i][:], wkt_d[i * P:(i + 1) * P, :])
                dma.dma_start(wvt[i][:], wvt_d[i * P:(i + 1) * P, :])
                dma.dma_start(wo1t[i][:], wo1t_d[i * P:(i + 1) * P, :])
                dma.dma_start(wo2t[i][:], wo2t_d[i * P:(i + 1) * P, :])
            bqb = wp.tile([P, C], f32, name="bqb")
            bkb = wp.tile([P, C], f32, name="bkb")
            dma.dma_start(bqb[:], bqb_d[:])
            dma.dma_start(bkb[:], bkb_d[:])
            vec = {}
            for nm, d in (("bv", bv_d), ("bns", bns_d), ("bnt", bnt_d),
                          ("bo1", bo1_d), ("bo2", bo2_d)):
                vec[nm] = [wp.tile([P, 1], f32, name=f"{nm}{i}") for i in range(2)]
                for i in range(2):
                    dma.dma_start(vec[nm][i][:], d[i * P:(i + 1) * P, None])
            ident = wp.tile([P, P], f32, name="ident")
            dma.dma_start(ident[:], id_d[:])

            value = [vp.tile([P, N], bf16, name=f"value{i}") for i in range(2)]
            attnT = [sp.tile([P, C], bf16, name=f"attnT{i}") for i in range(2)]

            with (
                tc.tile_pool(name="ps_s", bufs=1, space="PSUM") as ps_s,
                tc.tile_pool(name="xin", bufs=4) as xp,
                tc.tile_pool(name="qk", bufs=4) as qkp,
                tc.tile_pool(name="ps_a", bufs=2, space="PSUM") as psa,
            ):
                psum_s = [ps_s.tile([P, C], f32, name=f"psum_s{i}") for i in range(2)]
                pend = []

                def emit_scores(pair, nch):
                    qT, kT = pair
                    for cq in range(2):
                        nc.tensor.matmul(
                            psum_s[cq][:],
                            qT[:, cq * P:(cq + 1) * P], kT[:],
                            start=(nch == 0), stop=(nch == 71),
                            skip_group_check=True)

                for t in range(NT):
                    xq = [xp.tile([P, 512], bf16, name=f"xq{i}", tag=f"xq{i}")
                          for i in range(2)]
                    xv = [xp.tile([P, 512], bf16, name=f"xv{i}", tag=f"xv{i}")
                          for i in range(2)]
                    for i in range(2):
                        dma.dma_start(xq[i][:], qb_d[i * P:(i + 1) * P,
                                                     t * 512:(t + 1) * 512])
                        dma.dma_start(xv[i][:], vb_d[i * P:(i + 1) * P,
                                                     t * 512:(t + 1) * 512])
                    for d in range(2):
                        pv = psa.tile([P, 512], f32, name="pv", tag="pv")
                        nc.tensor.matmul(pv[:], wvt[0][:, d * P:(d + 1) * P],
                                         xq[0][:], start=True, stop=False)
                        nc.tensor.matmul(pv[:], wvt[1][:, d * P:(d + 1) * P],
                                         xq[1][:], start=False, stop=True)
                        nc.scalar.activation(value[d][:, t * 512:(t + 1) * 512],
                                             pv[:], AF.Identity,
                                             bias=vec["bv"][d][:])
                    for j in range(4):
                        nch = t * 4 + j
                        pq = psa.tile([P, C], f32, name="pq", tag="pq")
                        nc.tensor.matmul(pq[:], xq[0][:, j * P:(j + 1) * P],
                                         wqt[0][:], start=True, stop=False)
                        nc.tensor.matmul(pq[:], xq[1][:, j * P:(j + 1) * P],
                                         wqt[1][:], start=False, stop=True)
                        pk = psa.tile([P, C], f32, name="pk", tag="pk")
                        nc.tensor.matmul(pk[:], xv[0][:, j * P:(j + 1) * P],
                                         wkt[0][:], start=True, stop=False)
                        nc.tensor.matmul(pk[:], xv[1][:, j * P:(j + 1) * P],
                                         wkt[1][:], start=False, stop=True)
                        qT = qkp.tile([P, C], bf16, name="qT", tag="qT")
                        kT = qkp.tile([P, C], bf16, name="kT", tag="kT")
                        nc.vector.tensor_add(qT[:], pq[:], bqb[:])
                        nc.vector.tensor_add(kT[:], pk[:], bkb[:])
                        pend.append((qT, kT))
                        if len(pend) > 2:
                            emit_scores(pend.pop(0), nch - 2)
                for i, pair in enumerate(pend):
                    emit_scores(pair, 70 + i)

                sx = qkp
                pst = psa
                attn2 = []
                for cq in range(2):
                    negmax = sx.tile([P, 1], f32, name="negmax", tag=f"nm{cq}")
                    nc.vector.reduce_max(negmax[:], psum_s[cq][:], axis=AX.X,
                                         negate=True)
                    att = sx.tile([P, C], f32, name="att", tag=f"att{cq}")
                    sume = sx.tile([P, 1], f32, name="sume", tag=f"se{cq}")
                    nc.scalar.activation(att[:], psum_s[cq][:], AF.Exp,
                                         bias=negmax[:], accum_out=sume[:])
                    recip = sx.tile([P, 1], f32, name="recip", tag=f"rc{cq}")
                    nc.vector.reciprocal(recip[:], sume[:])
                    a2 = sx.tile([P, C], f32, name="a2", tag=f"a2{cq}")
                    nc.scalar.activation(a2[:], att[:], AF.Copy, scale=recip[:])
                    attn2.append(a2)
                for j in range(2):
                    for i in range(2):
                        nc.tensor.transpose(psum_s[j][:, i * P:(i + 1) * P],
                                            attn2[i][:, j * P:(j + 1) * P],
                                            ident[:])
                    nc.vector.tensor_copy(attnT[j][:], psum_s[j][:])

            with (
                tc.tile_pool(name="yb", bufs=3) as yb,
                tc.tile_pool(name="ps_b", bufs=2, space="PSUM") as psb,
            ):
                vv = [value[d][:].rearrange("p (c k) -> p c k", k=KB)
                      for d in range(2)]
                for kk in range(NT):
                    ys = []
                    for cp in range(2):
                        po = psb.tile([P, 512], f32, name="po", tag=f"po{cp}")
                        for ki in range(2):
                            k = 2 * kk + ki
                            for d in range(2):
                                nc.tensor.matmul(
                                    po[:, ki * C:(ki + 1) * C],
                                    vv[d][:, cp * P:(cp + 1) * P, k],
                                    attnT[d][:],
                                    start=(d == 0), stop=(d == 1))
                        y = yb.tile([P, 512], bf16, name="y", tag=f"y{cp}")
                        nc.scalar.activation(y[:], po[:], AF.Lrelu,
                                             bias=vec["bnt"][cp][:],
                                             scale=vec["bns"][cp][:],
                                             alpha=ALPHA)
                        ys.append(y)
                    hs = []
                    for o in range(2):
                        ph = psb.tile([P, 512], f32, name="ph", tag=f"ph{o}", bufs=1)
                        nc.tensor.matmul(ph[:], wo1t[0][:, o * P:(o + 1) * P],
                                         ys[0][:], start=True, stop=False)
                        nc.tensor.matmul(ph[:], wo1t[1][:, o * P:(o + 1) * P],
                                         ys[1][:], start=False, stop=True)
                        h = yb.tile([P, 512], bf16, name="h", tag=f"h{o}")
                        nc.scalar.activation(h[:], ph[:], AF.Lrelu,
                                             bias=vec["bo1"][o][:], alpha=ALPHA)
                        hs.append(h)
                    for o2 in range(2):
                        pf = psb.tile([P, 512], f32, name="pf", tag=f"pf{o2}", bufs=1)
                        nc.tensor.matmul(pf[:], wo2t[0][:, o2 * P:(o2 + 1) * P],
                                         hs[0][:], start=True, stop=False)
                        nc.tensor.matmul(pf[:], wo2t[1][:, o2 * P:(o2 + 1) * P],
                                         hs[1][:], start=False, stop=True)
                        ob = yb.tile([P, 512], f32, name="ob", tag=f"ob{o2}")
                        nc.vector.tensor_scalar(ob[:], pf[:], vec["bo2"][o2][:],
                                                None, op0=mybir.AluOpType.add)
                        dma.dma_start(out_d[o2 * P:(o2 + 1) * P,
                                            kk * 512:(kk + 1) * 512], ob[:])
    return nc


def _split_waits(nc):
    for f in nc.m.functions:
        for bb in f.blocks:
            new = []
            for inst in bb.instructions:
                si = inst.sync_info
                if (si is not None and si.on_wait and len(si.on_wait) > 1
                        and not isinstance(inst, (mybir.InstNoOp,
                                                  mybir.InstEventSemaphore))):
                    for wi, w in enumerate(si.on_wait[:-1]):
                        new.append(mybir.InstNoOp(
                            name=f"{inst.name}-ws{wi}",
                            ins=[], outs=[],
                            engine=inst.engine,
                            sync_info=mybir.SyncInfo(on_wait=[w], on_update=[]),
                            bass_nofuse=True,
                        ))
                    inst.sync_info = mybir.SyncInfo(on_wait=[si.on_wait[-1]],
                                                    on_update=list(si.on_update))
                new.append(inst)
            bb.instructions[:] = new


def _prep(inputs):
    f = np.float32
    bb = ml_dtypes.bfloat16
    scale = f(1.0) / f(np.sqrt(N))
    wqt = (inputs["Wq"].T.astype(f) * scale).astype(bb)
    wkt = inputs["Wk"].T.astype(f).astype(bb)
    wvt = inputs["Wv"].T.astype(f).astype(bb)
    wo1t = inputs["Wo1"].T.astype(f).astype(bb)
    wo2t = inputs["Wo2"].T.astype(f).astype(bb)
    bqb = np.tile((inputs["bq"].astype(f) * scale)[None, :], (P, 1)).astype(f)
    bkb = np.tile(inputs["bk"].astype(f)[None, :], (P, 1)).astype(f)
    bns = (inputs["bn_gamma"].astype(f)
           / np.sqrt(inputs["bn_var"].astype(f) + np.float32(1e-4))).astype(f)
    bnt = (inputs["bn_beta"].astype(f)
           - inputs["bn_mean"].astype(f) * bns).astype(f)
    common = {
        "wqt": np.ascontiguousarray(wqt), "wkt": np.ascontiguousarray(wkt),
        "wvt": np.ascontiguousarray(wvt), "wo1t": np.ascontiguousarray(wo1t),
        "wo2t": np.ascontiguousarray(wo2t),
        "bqb": bqb, "bkb": bkb,
        "bv": inputs["bv"].astype(f), "bns": bns, "bnt": bnt,
        "bo1": inputs["bo1"].astype(f), "bo2": inputs["bo2"].astype(f),
        "ident": np.eye(P, dtype=f),
    }
    q = np.asarray(inputs["q"], dtype=f).reshape(B, C, N).astype(bb)
    v = np.asarray(inputs["v"], dtype=f).reshape(B, C, N).astype(bb)
    in_maps = []
    for b in range(B):
        m = dict(common)
        m["qb"] = np.ascontiguousarray(q[b])
        m["vb"] = np.ascontiguousarray(v[b])
        in_maps.append(m)
    return in_maps


def kernel(_trace=False, **inputs):
    if "nc" not in _cached:
        nc = _build()
        _split_waits(nc)
        _cached["nc"] = nc
    nc = _cached["nc"]
    in_maps = _prep(inputs)
    res = run_bass_kernel_spmd(nc, in_maps, core_ids=list(range(B)),
                               trace=_trace)
    out = np.stack([res.results[b]["out"] for b in range(B)], axis=0)
    if _trace:
        kernel.last_results = res
    return out.reshape(B, C, HH, WW).astype(np.float32)
